# revision 8
# baseline (speedup 1.0000x reference)
"""Trainium2 Bass kernel for nn_EventAwareVisual (8-core data parallel).

Strategy (per core, 4 batches):
  - Only g_e_v needs the full visual pipeline. Because the cross-attention
    query rows are all identical (broadcast event token), h_v_mean collapses
    exactly to cq @ mha_Wov, so g_v / output_local_v depend only on the event
    features (tiny compute).
  - The visual pipeline is computed in transposed activation layout
    (features on partitions, sequence on free dim), with the self-attention
    probabilities never normalized or multiplied by V: instead we compute
    p_{h,h'} = (ca_h / Z_h')^T expS_h'  and r_{h,h'} = p_{h,h'} @ h_f0, then
    h_e_mean = sum_p r_p @ G_p with G_p host-precomputed weight products.
  - Attention head matmuls (K=32 / M=4) are packed with tile_position so 4
    heads run concurrently on the PE array.
"""

import math

import numpy as np

import concourse.bass as bass
import concourse.tile as tile
from concourse import bacc, mybir
from concourse.bass_utils import run_bass_kernel_spmd

F32 = mybir.dt.float32
AF = mybir.ActivationFunctionType
OP = mybir.AluOpType

D = 128       # d_model
H = 4         # heads
DH = 32       # head dim
S = 512       # visual seq len
DV = 512      # visual feature dim
DT = 512      # event feature dim
BPC = 4       # batches per core
NCORES = 8
EPS = 1e-5
SCALE = 1.0 / math.sqrt(DH)

_CACHE = {}


def _build_nc(debug=False):
    nc = bacc.Bacc("TRN2", target_bir_lowering=False)

    # ---- DRAM I/O ----
    visual = nc.dram_tensor("visual", [BPC, S, DV], F32, kind="ExternalInput")
    event = nc.dram_tensor("event", [BPC, DT], F32, kind="ExternalInput")
    Wec = nc.dram_tensor("Wec", [DT, D], F32, kind="ExternalInput")
    Wvc = nc.dram_tensor("Wvc", [DV, D], F32, kind="ExternalInput")
    w_names = ["W_bot", "e_top", "mmaWq", "mmaWk", "mhaWq", "mhaWov",
               "WkT_s", "W_ge", "W_gv", "W_c1", "ident"]
    wd = {n: nc.dram_tensor(n, [D, D], F32, kind="ExternalInput") for n in w_names}
    W_c2 = nc.dram_tensor("W_c2", [D, 2], F32, kind="ExternalInput")
    G_d = nc.dram_tensor("G_all", [16, D, D], F32, kind="ExternalInput")
    hmask_d = nc.dram_tensor("headmask", [D, H], F32, kind="ExternalInput")
    cols_d = {n: nc.dram_tensor(n, [D, 1], F32, kind="ExternalInput")
              for n in ["b_e_col", "b_v_col", "b_projT_col"]}
    row_names = ["b_ge_row", "b_gv_row", "b_c1_row", "lneg", "lneb",
                 "lnvg", "lnvb"]
    rows_d = {n: nc.dram_tensor(n, [BPC, D], F32, kind="ExternalInput")
              for n in row_names}
    b_c2_d = nc.dram_tensor("b_c2_row", [BPC, 2], F32, kind="ExternalInput")
    eps_d = nc.dram_tensor("eps_row", [BPC, 1], F32, kind="ExternalInput")

    o_gev = nc.dram_tensor("g_e_v", [BPC, D], F32, kind="ExternalOutput")
    o_gv = nc.dram_tensor("g_v", [BPC, D], F32, kind="ExternalOutput")
    o_loc = nc.dram_tensor("out_local", [BPC, 2], F32, kind="ExternalOutput")
    dbg = {}
    if debug:
        for n, shp in [("d_hvT", [D, S]), ("d_hf0T", [D, S]), ("d_qT", [D, S]),
                       ("d_kT", [D, S]), ("d_Z", [D, 16]), ("d_caT", [D, 16]),
                       ("d_p", [D, S]), ("d_r", [D, D]), ("d_Rall", [D, 64]),
                       ("d_hem", [BPC, D]), ("d_expS0", [D, S]),
                       ("d_vT0", [D, S])]:
            dbg[n] = nc.dram_tensor(n, shp, F32, kind="ExternalOutput")

    with tile.TileContext(nc) as tc:
        with (
            tc.tile_pool(name="wpool", bufs=1) as wp,
            tc.tile_pool(name="evp", bufs=1) as evp,
            tc.tile_pool(name="vpool", bufs=2) as vpool,
            tc.tile_pool(name="vtpool", bufs=2) as vtpool,
            tc.tile_pool(name="actp", bufs=2) as actp,
            tc.tile_pool(name="expp", bufs=1) as expp,
            tc.tile_pool(name="smalls", bufs=2) as smalls,
            tc.tile_pool(name="ppool", bufs=2) as ppool,
            tc.tile_pool(name="finals", bufs=1) as fin,
            tc.tile_pool(name="pscore", bufs=1, space="PSUM") as pscore,
            tc.tile_pool(name="pmed", bufs=2, space="PSUM") as pmed,
            tc.tile_pool(name="pacc", bufs=1, space="PSUM") as pacc,  # 4 tags

        ):
            # ---- load weights/constants ----
            We_sb = wp.tile([D, 4, D], F32)
            nc.sync.dma_start(We_sb[:], Wec[:, :].rearrange("(c p) m -> p c m", p=D))
            Wv_sb = wp.tile([D, 4, D], F32)
            nc.sync.dma_start(Wv_sb[:], Wvc[:, :].rearrange("(c p) m -> p c m", p=D))
            ws = {}
            for n in w_names:
                ws[n] = wp.tile([D, D], F32, tag=f"w_{n}", name=f"w_{n}")
                nc.sync.dma_start(ws[n][:], wd[n][:, :])
            Wc2_sb = wp.tile([D, 2], F32)
            nc.sync.dma_start(Wc2_sb[:], W_c2[:, :])
            G_sb = wp.tile([D, 16, D], F32)
            nc.sync.dma_start(G_sb[:], G_d[:, :, :].rearrange("g k n -> k g n"))
            hmask = wp.tile([D, H], F32)
            nc.sync.dma_start(hmask[:], hmask_d[:, :])
            cols = {}
            for n in cols_d:
                cols[n] = wp.tile([D, 1], F32, tag=f"c_{n}", name=f"c_{n}")
                nc.sync.dma_start(cols[n][:], cols_d[n][:, :])
            rows = {}
            for n in row_names:
                rows[n] = wp.tile([BPC, D], F32, tag=f"r_{n}", name=f"r_{n}")
                nc.sync.dma_start(rows[n][:], rows_d[n][:, :])
            bc2_sb = wp.tile([BPC, 2], F32)
            nc.sync.dma_start(bc2_sb[:], b_c2_d[:, :])
            eps_sb = wp.tile([BPC, 1], F32)
            nc.sync.dma_start(eps_sb[:], eps_d[:, :])
            ident = ws["ident"]

            # ---- event phase (all 4 batches at once) ----
            ev_sb = evp.tile([BPC, DT], F32)
            nc.sync.dma_start(ev_sb[:], event[:, :])
            ps_efT = pmed.tile([D, 512], F32, tag="pm")
            for c in range(4):
                nc.tensor.transpose(ps_efT[:, 4 * c:4 * c + 4],
                                    ev_sb[:, 128 * c:128 * c + 128],
                                    ident[:BPC, :BPC])
            efT = evp.tile([D, 16], F32)
            nc.vector.tensor_copy(efT[:], ps_efT[:, :16])
            ps_he = pmed.tile([D, 512], F32, tag="pm")
            for c in range(4):
                nc.tensor.matmul(ps_he[:, :BPC], We_sb[:, c, :], efT[:, 4 * c:4 * c + 4],
                                 start=(c == 0), stop=(c == 3))
            h_eT = evp.tile([D, BPC], F32)
            nc.vector.tensor_scalar(h_eT[:], ps_he[:, :BPC], cols["b_e_col"], 0.0,
                                    op0=OP.add, op1=OP.max)
            # e_contribT = e_top^T h_e + b_proj
            ps_ec = pmed.tile([D, 512], F32, tag="pm")
            nc.tensor.matmul(ps_ec[:, :BPC], ws["e_top"], h_eT[:], start=True, stop=True)
            e_contribT = evp.tile([D, BPC], F32)
            nc.vector.tensor_scalar(e_contribT[:], ps_ec[:, :BPC],
                                    cols["b_projT_col"], None, op0=OP.add)
            # cqT
            ps_cq = pmed.tile([D, 512], F32, tag="pm")
            nc.tensor.matmul(ps_cq[:, :BPC], ws["mhaWq"], h_eT[:], start=True, stop=True)
            cqT = evp.tile([D, BPC], F32)
            nc.vector.tensor_copy(cqT[:], ps_cq[:, :BPC])
            # Cq blocks and U
            cqblk = evp.tile([D, 16], F32)
            for b in range(BPC):
                nc.vector.tensor_scalar_mul(cqblk[:, 4 * b:4 * b + 4], hmask[:],
                                            cqT[:, b:b + 1])
            ps_U = pmed.tile([D, 512], F32, tag="pm")
            nc.tensor.matmul(ps_U[:, :16], ws["WkT_s"], cqblk[:], start=True, stop=True)
            U_sb = evp.tile([D, 16], F32)
            nc.vector.tensor_copy(U_sb[:], ps_U[:, :16])
            # h_v_meanT = mha_Wov^T cq
            ps_hm = pmed.tile([D, 512], F32, tag="pm")
            nc.tensor.matmul(ps_hm[:, :BPC], ws["mhaWov"], cqT[:], start=True, stop=True)
            hvmT = evp.tile([D, BPC], F32)
            nc.vector.tensor_copy(hvmT[:], ps_hm[:, :BPC])

            R_all = fin.tile([D, 64], F32)

            # ---- per-batch visual pipeline ----
            for b in range(BPC):
                vrow = vpool.tile([D, 4, DV], F32, tag="vrow")
                nc.sync.dma_start(vrow[:],
                                  visual[b].rearrange("(t p) d -> p t d", p=D))
                vT = vtpool.tile([D, 4, S], F32, tag="vT")
                for c in range(4):
                    ps_v = pmed.tile([D, 512], F32, tag="pm")
                    for t in range(4):
                        nc.tensor.transpose(ps_v[:, 128 * t:128 * t + 128],
                                            vrow[:, t, 128 * c:128 * c + 128],
                                            ident[:])
                    nc.vector.tensor_copy(vT[:, c, :], ps_v[:])
                # h_vT = relu(W_v^T V^T + b_v)
                ps_hv = pmed.tile([D, 512], F32, tag="pm")
                for c in range(4):
                    nc.tensor.matmul(ps_hv[:], Wv_sb[:, c, :], vT[:, c, :],
                                     start=(c == 0), stop=(c == 3))
                h_vT = actp.tile([D, S], F32, tag="hv")
                nc.vector.tensor_scalar(h_vT[:], ps_hv[:], cols["b_v_col"], 0.0,
                                        op0=OP.add, op1=OP.max)
                # h_f0T = W_bot^T h_vT + e_contrib (no relu)
                ps_hf = pmed.tile([D, 512], F32, tag="pm")
                nc.tensor.matmul(ps_hf[:], ws["W_bot"], h_vT[:], start=True, stop=True)
                h_f0T = actp.tile([D, S], F32, tag="hf0")
                nc.vector.tensor_scalar(h_f0T[:], ps_hf[:], e_contribT[:, b:b + 1],
                                        None, op0=OP.add)
                # qT, kT
                ps_q = pmed.tile([D, 512], F32, tag="pm")
                nc.tensor.matmul(ps_q[:], ws["mmaWq"], h_f0T[:], start=True, stop=True)
                qT = actp.tile([D, S], F32, tag="qT")
                nc.vector.tensor_copy(qT[:], ps_q[:])
                ps_k = pmed.tile([D, 512], F32, tag="pm")
                nc.tensor.matmul(ps_k[:], ws["mmaWk"], h_f0T[:], start=True, stop=True)
                kT = actp.tile([D, S], F32, tag="kT")
                nc.vector.tensor_copy(kT[:], ps_k[:])
                if debug and b == 0:
                    nc.sync.dma_start(dbg["d_hvT"][:, :], h_vT[:])
                    nc.sync.dma_start(dbg["d_hf0T"][:, :], h_f0T[:])
                    nc.sync.dma_start(dbg["d_qT"][:, :], qT[:])
                    nc.sync.dma_start(dbg["d_kT"][:, :], kT[:])
                    nc.sync.dma_start(dbg["d_vT0"][:, :], vT[:, 0, :])
                # cross-attention row softmax (scale folded into WkT_s)
                ps_cs = pmed.tile([D, 512], F32, tag="pm")
                nc.tensor.matmul(ps_cs[:BPC, :], U_sb[:, 4 * b:4 * b + 4], h_vT[:],
                                 start=True, stop=True)
                can_raw = smalls.tile([BPC, S], F32, tag="can_raw")
                casum = smalls.tile([BPC, 1], F32, tag="casum")
                nc.scalar.activation(can_raw[:], ps_cs[:BPC, :], AF.Exp,
                                     accum_out=casum[:])
                carec = smalls.tile([BPC, 1], F32, tag="carec")
                nc.vector.reciprocal(carec[:], casum[:])
                can = smalls.tile([BPC, S], F32, tag="can")
                nc.vector.tensor_scalar_mul(can[:], can_raw[:], carec[:])
                ps_caT = pmed.tile([D, 512], F32, tag="pm")
                for t in range(4):
                    nc.tensor.transpose(ps_caT[:, 4 * t:4 * t + 4],
                                        can[:, 128 * t:128 * t + 128],
                                        ident[:BPC, :BPC])
                caT = smalls.tile([D, 16], F32, tag="caT")
                nc.vector.tensor_copy(caT[:], ps_caT[:, :16])

                # self-attention: scores -> exp (+Z) -> c -> p accumulation
                Z = smalls.tile([D, 16], F32, tag="Z")
                Zinv = smalls.tile([D, 16], F32, tag="Zi")
                expS = expp.tile([D, 4, H * S], F32, tag="expS")
                pps = [pacc.tile([D, 512], F32, tag=f"pp{hp}", name=f"pp{hp}")
                       for hp in range(H)]
                for t in range(4):
                    for half in range(2):
                        ps_s = pscore.tile([D, 2 * S], F32, tag="scores")
                        for i in range(2):
                            h = 2 * half + i
                            nc.tensor.matmul(ps_s[:, 512 * i:512 * i + 512],
                                             qT[32 * h:32 * h + 32,
                                                128 * t:128 * t + 128],
                                             kT[32 * h:32 * h + 32, :],
                                             start=True, stop=True,
                                             tile_position=(32 * h, 0))
                        for i in range(2):
                            h = 2 * half + i
                            nc.scalar.activation(expS[:, t, 512 * h:512 * h + 512],
                                                 ps_s[:, 512 * i:512 * i + 512],
                                                 AF.Exp, scale=SCALE,
                                                 accum_out=Z[:, 4 * t + h:4 * t + h + 1])
                    nc.vector.reciprocal(Zinv[:, 4 * t:4 * t + 4], Z[:, 4 * t:4 * t + 4])
                    c_t = smalls.tile([D, 16], F32, tag="ct")
                    for hp in range(H):
                        nc.vector.tensor_scalar_mul(c_t[:, 4 * hp:4 * hp + 4],
                                                    caT[:, 4 * t:4 * t + 4],
                                                    Zinv[:, 4 * t + hp:4 * t + hp + 1])
                    for hp in range(H):
                        nc.tensor.matmul(pps[hp][0:4, :],
                                         c_t[:, 4 * hp:4 * hp + 4],
                                         expS[:, t, 512 * hp:512 * hp + 512],
                                         start=(t == 0), stop=(t == 3))
                # p tail: r = p @ h_f0, scatter into R_all
                p_sb = ppool.tile([4, 4, 512], F32, tag="p")
                for hp in range(H):
                    nc.vector.tensor_copy(p_sb[0:4, hp, :], pps[hp][0:4, :])
                ps_pT = pmed.tile([D, 512], F32, tag="pm")
                for u in range(4):
                    for hp in range(H):
                        nc.tensor.transpose(
                            ps_pT[:, 16 * u + 4 * hp:16 * u + 4 * hp + 4],
                            p_sb[0:4, hp, 128 * u:128 * u + 128], ident[:4, :4])
                pT_sb = ppool.tile([D, 64], F32, tag="pT")
                nc.vector.tensor_copy(pT_sb[:], ps_pT[:, :64])
                ps_hr = pmed.tile([D, 512], F32, tag="pm")
                for u in range(4):
                    nc.tensor.transpose(ps_hr[:, 128 * u:128 * u + 128],
                                        h_f0T[:, 128 * u:128 * u + 128], ident[:])
                hrows = ppool.tile([D, 512], F32, tag="hr")
                nc.vector.tensor_copy(hrows[:], ps_hr[:])
                ps_r = pmed.tile([D, 512], F32, tag="pm")
                for u in range(4):
                    nc.tensor.matmul(ps_r[:16, :128], pT_sb[:, 16 * u:16 * u + 16],
                                     hrows[:, 128 * u:128 * u + 128],
                                     start=(u == 0), stop=(u == 3))
                r_sb = ppool.tile([16, D], F32, tag="r")
                nc.vector.tensor_copy(r_sb[:], ps_r[:16, :128])
                if debug and b == 0:
                    nc.sync.dma_start(dbg["d_Z"][:, :], Z[:])
                    nc.sync.dma_start(dbg["d_caT"][:, :], caT[:])
                    for hp in range(H):
                        nc.sync.dma_start(dbg["d_p"][4 * hp:4 * hp + 4, :],
                                          p_sb[0:4, hp, :])
                    nc.sync.dma_start(dbg["d_r"][:16, :], r_sb[:])
                    nc.sync.dma_start(dbg["d_expS0"][:, :], expS[:, 0, :512])
                ps_rT = pmed.tile([D, 512], F32, tag="pm")
                nc.tensor.transpose(ps_rT[:, :16], r_sb[:], ident[:16, :16])
                # scatter: R_all[:, 4*p + b] = ps_rT[:, p]
                dst = R_all[:, :].rearrange("q (g f) -> q g f", f=4)[:, :, b]
                nc.vector.tensor_copy(dst, ps_rT[:, :16])

            # ---- final phase ----
            ps_hem = pmed.tile([D, 512], F32, tag="pm")
            for p in range(16):
                nc.tensor.matmul(ps_hem[:BPC, :128], R_all[:, 4 * p:4 * p + 4],
                                 G_sb[:, p, :], start=(p == 0), stop=(p == 15))
            hem = fin.tile([BPC, D], F32, tag="hem")
            nc.vector.tensor_copy(hem[:], ps_hem[:BPC, :128])
            if debug:
                nc.sync.dma_start(dbg["d_Rall"][:, :], R_all[:])
                nc.sync.dma_start(dbg["d_hem"][:, :], hem[:])

            def rows_linear(xrows, W_sb, tag):
                """[4,128] rows @ W -> [4,128] rows (via transpose + matmul)."""
                ps_t = pmed.tile([D, 512], F32, tag="pm")
                nc.tensor.transpose(ps_t[:, :BPC], xrows[:], ident[:BPC, :BPC])
                xT = fin.tile([D, BPC], F32, tag=f"{tag}_T")
                nc.vector.tensor_copy(xT[:], ps_t[:, :BPC])
                ps_o = pmed.tile([D, 512], F32, tag="pm")
                nc.tensor.matmul(ps_o[:BPC, :128], xT[:], W_sb[:], start=True, stop=True)
                return ps_o

            def layernorm(xrows, gain, bias, outrows):
                stats = fin.tile([BPC, 6], F32, tag="stats")
                nc.vector.bn_stats(stats[:], xrows[:])
                mv = fin.tile([BPC, 2], F32, tag="mv")
                nc.vector.bn_aggr(mv[:], stats[:])
                std = fin.tile([BPC, 1], F32, tag="std")
                nc.scalar.activation(std[:], mv[:, 1:2], AF.Sqrt, bias=eps_sb[:])
                rstd = fin.tile([BPC, 1], F32, tag="rstd")
                nc.vector.reciprocal(rstd[:], std[:])
                xn = fin.tile([BPC, D], F32, tag="xn")
                nc.vector.tensor_scalar(xn[:], xrows[:], mv[:, 0:1], rstd[:],
                                        op0=OP.subtract, op1=OP.mult)
                nc.vector.tensor_mul(xn[:], xn[:], gain[:])
                nc.vector.tensor_add(outrows[:], xn[:], bias[:])

            # g_e_v branch
            ps_ze = pmed.tile([D, 512], F32, tag="pm")
            nc.tensor.transpose(ps_ze[:, :BPC], hem[:], ident[:BPC, :BPC])
            hemT = fin.tile([D, BPC], F32, tag="hemT")
            nc.vector.tensor_copy(hemT[:], ps_ze[:, :BPC])
            ps_ze2 = pmed.tile([D, 512], F32, tag="pm")
            nc.tensor.matmul(ps_ze2[:BPC, :128], hemT[:], ws["W_ge"],
                             start=True, stop=True)
            ze = fin.tile([BPC, D], F32, tag="ze")
            nc.vector.tensor_add(ze[:], ps_ze2[:BPC, :128], rows["b_ge_row"][:])
            nc.vector.tensor_scalar_max(ze[:], ze[:], 0.0)
            gev_rows = fin.tile([BPC, D], F32, tag="gev")
            layernorm(ze, rows["lneg"], rows["lneb"], gev_rows)
            nc.sync.dma_start(o_gev[:, :], gev_rows[:])

            # g_v branch (event-only)
            ps_zv = pmed.tile([D, 512], F32, tag="pm")
            nc.tensor.matmul(ps_zv[:BPC, :128], hvmT[:], ws["W_gv"],
                             start=True, stop=True)
            zv = fin.tile([BPC, D], F32, tag="zv")
            nc.vector.tensor_add(zv[:], ps_zv[:BPC, :128], rows["b_gv_row"][:])
            nc.vector.tensor_scalar_max(zv[:], zv[:], 0.0)
            gv_rows = fin.tile([BPC, D], F32, tag="gvr")
            layernorm(zv, rows["lnvg"], rows["lnvb"], gv_rows)
            nc.sync.dma_start(o_gv[:, :], gv_rows[:])

            # classifier: relu(g_v @ W_c1 + b_c1) @ W_c2 + b_c2
            ps_z1 = rows_linear(gv_rows, ws["W_c1"], "z1")
            z1 = fin.tile([BPC, D], F32, tag="z1")
            nc.vector.tensor_add(z1[:], ps_z1[:BPC, :128], rows["b_c1_row"][:])
            nc.vector.tensor_scalar_max(z1[:], z1[:], 0.0)
            ps_z1t = pmed.tile([D, 512], F32, tag="pm")
            nc.tensor.transpose(ps_z1t[:, :BPC], z1[:], ident[:BPC, :BPC])
            z1T = fin.tile([D, BPC], F32, tag="z1T")
            nc.vector.tensor_copy(z1T[:], ps_z1t[:, :BPC])
            ps_out = pmed.tile([D, 512], F32, tag="pm")
            nc.tensor.matmul(ps_out[:BPC, :2], z1T[:], Wc2_sb[:], start=True, stop=True)
            outl = fin.tile([BPC, 2], F32, tag="outl")
            nc.vector.tensor_add(outl[:], ps_out[:BPC, :2], bc2_sb[:])
            nc.sync.dma_start(o_loc[:, :], outl[:])

    nc.compile()
    return nc


def _host_prep(inputs):
    """Host-side weight-only preprocessing (shared across cores)."""
    f = lambda x: np.ascontiguousarray(np.asarray(x, dtype=np.float32))
    W_proj = f(inputs["W_proj"])
    mmaWo, mhaWv = f(inputs["mma_Wo"]), f(inputs["mha_Wv"])
    mmaWv, mhaWo = f(inputs["mma_Wv"]), f(inputs["mha_Wo"])
    M = mmaWo @ mhaWv
    G = np.zeros((16, D, D), np.float32)
    for hp in range(H):
        for h in range(H):
            hb = slice(h * DH, (h + 1) * DH)
            hpb = slice(hp * DH, (hp + 1) * DH)
            G[4 * hp + h] = mmaWv[:, hpb] @ M[hpb, hb] @ mhaWo[hb, :]
    hmask = np.zeros((D, H), np.float32)
    for h in range(H):
        hmask[h * DH:(h + 1) * DH, h] = 1.0
    rep = lambda v: np.ascontiguousarray(np.tile(f(v)[None, :], (BPC, 1)))
    wm = {
        "Wec": f(inputs["W_e"]),
        "Wvc": f(inputs["W_v"]),
        "W_bot": f(W_proj[D:]),
        "e_top": f(W_proj[:D]),
        "mmaWq": f(inputs["mma_Wq"]),
        "mmaWk": f(inputs["mma_Wk"]),
        "mhaWq": f(inputs["mha_Wq"]),
        "mhaWov": f(inputs["mha_Wov"]),
        "WkT_s": np.ascontiguousarray(f(inputs["mha_Wk"]).T / math.sqrt(DH)),
        "W_ge": f(inputs["W_ge"]),
        "W_gv": f(inputs["W_gv"]),
        "W_c1": f(inputs["W_c1"]),
        "ident": np.eye(D, dtype=np.float32),
        "W_c2": f(inputs["W_c2"]),
        "G_all": G,
        "headmask": hmask,
        "b_e_col": f(inputs["b_e"]).reshape(D, 1),
        "b_v_col": f(inputs["b_v"]).reshape(D, 1),
        "b_projT_col": f(inputs["b_proj"]).reshape(D, 1),
        "b_ge_row": rep(inputs["b_ge"]),
        "b_gv_row": rep(inputs["b_gv"]),
        "b_c1_row": rep(inputs["b_c1"]),
        "lneg": rep(inputs["ln_e_g"]),
        "lneb": rep(inputs["ln_e_b"]),
        "lnvg": rep(inputs["ln_v_g"]),
        "lnvb": rep(inputs["ln_v_b"]),
        "b_c2_row": np.ascontiguousarray(np.tile(f(inputs["b_c2"])[None, :], (BPC, 1))),
        "eps_row": np.full((BPC, 1), EPS, np.float32),
    }
    return wm


def kernel(**inputs):
    if "nc" not in _CACHE:
        _CACHE["nc"] = _build_nc()
    nc = _CACHE["nc"]
    wm = _host_prep(inputs)
    visual = np.ascontiguousarray(np.asarray(inputs["visual_fea"], np.float32))
    event = np.ascontiguousarray(
        np.asarray(inputs["event_fea"], np.float32)[:, 0, :])
    B = visual.shape[0]
    assert B == BPC * NCORES
    in_maps = []
    for c in range(NCORES):
        m = dict(wm)
        m["visual"] = np.ascontiguousarray(visual[c * BPC:(c + 1) * BPC])
        m["event"] = np.ascontiguousarray(event[c * BPC:(c + 1) * BPC])
        in_maps.append(m)
    res = run_bass_kernel_spmd(nc, in_maps, core_ids=list(range(NCORES)))
    out_local = np.concatenate([res.results[c]["out_local"] for c in range(NCORES)])
    g_v = np.concatenate([res.results[c]["g_v"] for c in range(NCORES)])
    g_e_v = np.concatenate([res.results[c]["g_e_v"] for c in range(NCORES)])
    return (out_local.astype(np.float32), g_v.astype(np.float32),
            g_e_v.astype(np.float32))
